# revision 31
# baseline (speedup 1.0000x reference)
"""Trainium2 Bass kernel: cubic B-spline upsampling x2 of a (2,3,96,96,96) volume.

Math: the reference op (recursive IIR prefilter along each spatial axis, then
an 8-tap stride-2 transposed conv along each axis) is linear and separable.
The whole per-axis operator is a dense 192x96 matrix M (built exactly on the
host in float64).  out = M (x) M (x) M applied along z, y, x.

Device strategy (8 NeuronCores, SPMD, no collectives):
  24 tasks = 6 (b,c) volumes x 4 z'-slices of 48 rows; 3 tasks per core.
  Per task, three matmul stages in "data-stationary" form (stationary operand
  = data tile, moving operand = spline matrix), chosen so NO transposes are
  needed and the final stage lands with (z',y')-chunks on partitions and x'
  on the free axis == the DRAM output layout:
    A: per x  (96 mms): lhsT = vol[z, x, :]   (96z,128y-pad) rhs=MzT(96,48)  -> (y, z')
    B: per z' (48 mms): lhsT = L1[y, z', :]   (96y,128x-pad) rhs=MT (96,192) -> (x, y')
    C: per 128-chunk of (z',y') (72 mms): lhsT = L2[x, c] (96x,128) rhs=MT   -> (chunk, x')
  Compute in bf16 (PSUM accumulates fp32); volume is pre-cast/transposed on
  host; output written as fp32.
"""

import math
import os
import sys

import numpy as np

for _p in ("/opt/trn_rl_repo",):
    if _p not in sys.path and os.path.isdir(_p):
        sys.path.insert(0, _p)

import ml_dtypes  # noqa: E402

BF16 = ml_dtypes.bfloat16

POLE = math.sqrt(3.0) - 2.0
GAIN = (1.0 - POLE) * (1.0 - 1.0 / POLE)  # 6.0
N = 96
F = 2
NOUT = N * F  # 192
NTASK_PER_CORE = 3
NCORES = 8
ZSLICE = NOUT // 4  # 48


def _cubic(t):
    a = np.abs(t)
    out = (2.0 / 3.0 + (0.5 * a - 1.0) * a**2) * (a < 1)
    out = out + (-((a - 2.0) ** 3) / 6.0) * ((a >= 1) & (a < 2))
    return out


def _prefilter_mat(n):
    """96x96 matrix of the causal+anticausal cubic-spline prefilter (float64)."""
    p = POLE
    xm = np.eye(n, dtype=np.float64) * GAIN
    i = np.arange(n)
    pows = p**i + p ** (2 * n - 1 - i)
    c = np.zeros((n, n), dtype=np.float64)
    c[0] = (pows @ xm) * (p / (1.0 - p ** (2 * n))) + xm[0]
    for k in range(1, n):
        c[k] = xm[k] + p * c[k - 1]
    out = np.zeros((n, n), dtype=np.float64)
    out[n - 1] = c[n - 1] * (p / (p - 1.0))
    for k in range(n - 2, -1, -1):
        out[k] = p * (out[k + 1] - c[k])
    return out


def _upsample_mat(n, f=F):
    """2n x n matrix of the edge-padded stride-2 transposed conv (float64)."""
    k = 4 * f  # f even -> is_odd == 0
    start = 1.0 / (2 * f) - 2.0
    pts = np.arange(k, dtype=np.float64) * (1.0 / f) + start
    ker = _cubic(pts)
    npad = n + 4
    U = np.zeros((f * n, npad), dtype=np.float64)
    for o in range(f * n):
        for i in range(npad):
            s = o + (k - 1) - f * i
            if 0 <= s < k:
                U[o, i] += ker[s]
    Uc = np.zeros((f * n, n), dtype=np.float64)
    for i in range(npad):
        j = min(max(i - 2, 0), n - 1)
        Uc[:, j] += U[:, i]
    return Uc


def build_M():
    """Exact 192x96 per-axis operator (float64)."""
    return _upsample_mat(N) @ _prefilter_mat(N)


_NC_CACHE = {}


def _strip_redundant_self_waits(nc):
    """Drop sem waits that are trivially satisfied by same-engine program order.

    Tile's per-proc wait emission is not transitively minimal: a PE matmul can
    end up waiting on the PE's own semaphore (already guaranteed by in-order
    engine execution) in addition to a cross-engine wait, and the MM ISA
    struct only has one sync-wait slot (walrus: "Too many sync wait
    commands"). A wait on sem S is redundant for instruction I on engine E iff
    S is only ever updated by E and the cumulative updates to S from E before
    I already reach the wait value.
    """
    import concourse.mybir as mybir

    for fn in nc.m.functions:
        for blk in fn.blocks:
            updaters = {}  # sem id -> set of engines updating it (block-wide)
            for i in blk.instructions:
                si = i.sync_info
                if si is None:
                    continue
                for u in si.on_update or []:
                    updaters.setdefault(u.id, set()).add(i.engine)
            seen = {}  # (engine, sem id) -> cumulative update count so far
            for i in blk.instructions:
                si = i.sync_info
                if si is None:
                    continue
                if si.on_wait:
                    kept = []
                    for w in si.on_wait:
                        if (
                            w.sync_type == "semaphore"
                            and w.wait_mode == "sem-ge-imm"
                            and updaters.get(w.id) == {i.engine}
                            and seen.get((i.engine, w.id), 0) >= w.wait_value
                        ):
                            continue  # implied by program order
                        kept.append(w)
                    if len(kept) != len(si.on_wait):
                        si.on_wait[:] = kept
                for u in si.on_update or []:
                    key = (i.engine, u.id)
                    seen[key] = seen.get(key, 0) + u.update_value
            # each engine ISA struct has a single sync-wait slot: offload
            # extra waits onto same-engine nops inserted just before
            new_insts = []
            nop_n = 0
            for i in blk.instructions:
                si = i.sync_info
                if si is not None and si.on_wait and len(si.on_wait) > 1:
                    extra = list(si.on_wait[:-1])
                    si.on_wait[:] = [si.on_wait[-1]]
                    for w in extra:
                        nop = mybir.InstNoOp(
                            name=f"I-waitnop-{blk.name}-{nop_n}", ins=[], outs=[]
                        )
                        nop_n += 1
                        nop.engine = i.engine
                        nop.sync_info = mybir.SyncInfo(on_wait=[w], on_update=[])
                        new_insts.append(nop)
                new_insts.append(i)
            if nop_n:
                blk.instructions[:] = new_insts


def _hoist_input_dmas(nc, n_hoist=11):
    """Move the first input DMAs ahead of the sync engine's entry barrier.

    The Tile/BSP prologue (entry EVSEM barrier + TENSOR_LOAD) delays the
    first dma_start by ~7us. The leading input DMAs have no waits (inputs
    are resident at NEFF start, dst tiles untouched), so issuing them first
    starts the HBM reads during the prologue.
    """
    import concourse.mybir as mybir

    blocks = nc.m.functions[0].blocks
    body = blocks[1]
    dmas = []
    for i in body.instructions:
        if type(i).__name__ == "InstDMACopy" and i.engine == mybir.EngineType.SP:
            si = i.sync_info
            if si is not None and si.on_wait:
                break  # stop at the first gated DMA
            dmas.append(i)
            if len(dmas) >= n_hoist:
                break
    if not dmas:
        return
    dset = set(id(x) for x in dmas)
    body.instructions[:] = [i for i in body.instructions if id(i) not in dset]
    # insert into the prologue block after the leading InstCall, ahead of
    # the entry barrier: the sync engine starts immediately, so these DMAs
    # issue at t~0 while the other engines are still loading their code
    pro = blocks[0].instructions
    pos = 1 if pro and type(pro[0]).__name__ == "InstCall" else 0
    pro[:] = pro[:pos] + dmas + pro[pos:]


def build_nc():
    import concourse.bass as bass
    import concourse.mybir as mybir
    from concourse.tile import TileContext

    bf16 = mybir.dt.bfloat16
    f32 = mybir.dt.float32

    nc = bass.Bass(enable_partition_id=False)
    vol_ext = nc.declare_dram_parameter("vol", [3, 128, 96 * 128], bf16, isOutput=False)
    mzt_ext = nc.declare_dram_parameter("mzt", [3, 128, 48], bf16, isOutput=False)
    mt_ext = nc.declare_dram_parameter("mt", [128, 192], bf16, isOutput=False)
    out_ext = nc.declare_dram_parameter("out", [3 * 9216, 192], bf16, isOutput=True)


    with TileContext(nc) as tc:
        with (
            tc.tile_pool(name="consts", bufs=1) as consts,
            tc.tile_pool(name="vols", bufs=3) as vols_pool,
            tc.tile_pool(name="l1", bufs=2) as l1_pool,
            tc.tile_pool(name="l2", bufs=2) as l2_pool,
            tc.tile_pool(name="stage", bufs=5) as stage_pool,
            tc.tile_pool(name="pab", bufs=2, space="PSUM") as pab_pool,
            tc.tile_pool(name="pc", bufs=4, space="PSUM") as pc_pool,
        ):
            mt = consts.tile([128, 192], bf16)
            nc.sync.dma_start(out=mt[:], in_=mt_ext[:])
            mzt_all = consts.tile([128, 3, 48], bf16)
            nc.sync.dma_start(out=mzt_all[:], in_=mzt_ext[:].transpose([1, 0, 2]))

            # software-pipelined task loop: stage C of task t-1 finishes
            # interleaved with stage A of task t so out-DMAs keep flowing
            vols = []
            L1s = {}
            L2s = {}
            carry = None  # (emit_c_group fn, next group idx) of previous task

            def load_vol(t):
                vol = vols_pool.tile([128, 96 * 128], bf16, name="vol")
                # graduated chunks: small early ones so stage A never starves
                bounds = [0, 1024, 2048, 3072, 4608, 6656, 9472, 12288]
                for ch in range(7):
                    nc.sync.dma_start(
                        out=vol[:, bounds[ch] : bounds[ch + 1]],
                        in_=vol_ext[t, :, bounds[ch] : bounds[ch + 1]],
                    )
                return vol

            def make_emit_c(t, L2f):
                stage_tiles = {}

                # finer DMAs on the last quarter of the last task so the
                # final drain is short
                def _layout(g):
                    if t < 2 or g < 27:
                        return divmod(g, 9) + (9, 0)
                    if g < 33:
                        q2, gq = divmod(g - 27, 3)
                        return 3 + q2, gq, 3, 6912
                    return 5 + (g - 33), 0, 1, 8448

                def emit_c_group(g):
                    q, gq, gper, base = _layout(g)
                    if gq == 0:
                        stage_tiles[q] = stage_pool.tile(
                            [128, 2 * gper, 192], bf16, name="stage"
                        )
                    stage = stage_tiles[q]
                    pc = pc_pool.tile([128, 2, 192], f32, name="pc")
                    for j in range(2):
                        ch = g * 2 + j
                        nc.tensor.matmul(
                            pc[:, j, :],
                            lhsT=L2f[:, ch * 128 : (ch + 1) * 128],
                            rhs=mt[:],
                            start=True,
                            stop=True,
                        )
                    if g in (17, 35):
                        nc.vector.tensor_copy(
                            stage[:, gq * 2 : gq * 2 + 2, :], pc[:, :, :]
                        )
                    else:
                        nc.scalar.copy(
                            stage[:, gq * 2 : gq * 2 + 2, :], pc[:, :, :]
                        )
                    if gq == gper - 1:
                        rows = 256 * gper
                        qoff = 0 if not base else (3 if gper == 3 else 5)
                        r0 = t * 9216 + base + (q - qoff) * rows
                        nc.sync.dma_start(
                            out=out_ext[r0 : r0 + rows, :].rearrange(
                                "(cl p) x -> p cl x", p=128
                            ),
                            in_=stage[:],
                        )

                return emit_c_group

            vols = [load_vol(0), load_vol(1), load_vol(2)]

            for t in range(3):
                vol = vols[t]

                # ---- stage A: contract z -> (y, z') per x ----
                # L1 layout (y, x, z'): stage-A copies are fully contiguous;
                # stage B's stationary reads x strided (LDW-side cost only)
                # L1 layout (y, z', x-pad128): B's stationary reads 128
                # contiguous cols; stage-A matmuls write PSUM strided
                # (x-minor) to match. y-rows 96:128 arrive as zeros via the
                # copies (zero-padded stationaries), only x-cols need memset.
                L1 = l1_pool.tile([128, 48, 128], bf16)
                nc.gpsimd.memset(L1[:, :, 96:128], 0.0)
                for g in range(12):  # groups of 8 x-slices -> one PSUM bank
                    pa = pab_pool.tile([128, 48, 8], f32, name="pa", tag="pab")
                    for j in range(8):
                        x = g * 8 + j
                        nc.tensor.matmul(
                            pa[:, :, j],
                            lhsT=vol[:, x * 128 : (x + 1) * 128],
                            rhs=mzt_all[:, t, :],
                            start=True,
                            stop=True,
                        )
                    # psum (96p, z':48, x:8) -> L1 (96p, z':48, x:8 at offset)
                    nc.vector.tensor_copy(
                        L1[:, :, g * 8 : (g + 1) * 8],
                        pa[:, :, :],
                    )
                    # previous task's held-back C groups ride along with A
                    if carry is not None:
                        c_emit, c_next = carry
                        c_emit(c_next)
                        carry = (c_emit, c_next + 1) if c_next + 1 < 36 else None
                while carry is not None:
                    c_emit, c_next = carry
                    c_emit(c_next)
                    carry = (c_emit, c_next + 1) if c_next + 1 < 36 else None

                # ---- stages B and C, interleaved ----
                # B (contract y -> (x, y') per z') fills L2 z'-rows in order;
                # C chunk c only needs z' <= ((c+1)*128-1)//192, so C groups
                # are emitted as soon as their rows exist; the last 4 groups
                # are held back to overlap the next task's stage A.
                L2 = l2_pool.tile([128, 48, 192], bf16)
                L2f = L2[:].rearrange("p a b -> p (a b)")  # (128, 9216)
                emit_c = make_emit_c(t, L2f)
                hold = 9 if t < 2 else 0
                g_next = 0
                for zz in range(12):  # B groups of 4 z' (2 PSUM banks)
                    pb = pab_pool.tile(
                        [128, 2, 2, 192], f32, name="pb", tag="pab",
                        padded_shape=[128, 2, 2, 256],
                    )
                    for b in range(2):
                        for jj in range(2):
                            zp = zz * 4 + b * 2 + jj
                            nc.tensor.matmul(
                                pb[:, b, jj, :],
                                lhsT=L1[:, zp, :],
                                rhs=mt[:],
                                start=True,
                                stop=True,
                            )
                    nc.vector.tensor_copy(
                        L2[:, zz * 4 : zz * 4 + 4, :].rearrange(
                            "p (b j) y -> p b j y", b=2
                        ),
                        pb[:, :, :, :],
                    )
                    # one B-group of slack so C's matmuls never catch the
                    # DVE copy that publishes their L2 rows
                    rows_done = (zz + 1) * 4 * 192 - 768
                    while (
                        g_next < 36 - hold and (g_next * 2 + 2) * 128 <= rows_done
                    ):
                        emit_c(g_next)
                        g_next += 1
                while g_next < 36 - hold:
                    emit_c(g_next)
                    g_next += 1
                carry = (emit_c, g_next) if hold else None
    _strip_redundant_self_waits(nc)
    _hoist_input_dmas(nc)
    return nc


def _task_map(core, t):
    gt = NTASK_PER_CORE * core + t
    bc, s = divmod(gt, 4)
    b, c = divmod(bc, 3)
    return b, c, s


def make_in_maps(volume, M):
    mt_b = np.zeros((128, 192), dtype=BF16)  # K-padded to 128 rows
    mt_b[:96] = np.ascontiguousarray(M.T).astype(BF16)
    in_maps = []
    for core in range(NCORES):
        vols = np.zeros((3, 128, 96, 128), dtype=BF16)
        mzts = np.zeros((3, 128, 48), dtype=BF16)
        for t in range(NTASK_PER_CORE):
            b, c, s = _task_map(core, t)
            vt = np.transpose(volume[b, c], (0, 2, 1))  # (z, x, y)
            vols[t, :96, :, :96] = vt.astype(BF16)
            mzts[t, :96] = np.ascontiguousarray(
                M[s * ZSLICE : (s + 1) * ZSLICE, :].T
            ).astype(BF16)
        in_maps.append(
            {"vol": vols.reshape(3, 128, 96 * 128), "mzt": mzts, "mt": mt_b}
        )
    return in_maps


def gather_out(results):
    out = np.zeros((2, 3, 192, 192, 192), dtype=np.float32)
    for core in range(NCORES):
        o = np.asarray(results[core]["out"], dtype=np.float32).reshape(3, 48, 192, 192)
        for t in range(NTASK_PER_CORE):
            b, c, s = _task_map(core, t)
            out[b, c, s * ZSLICE : (s + 1) * ZSLICE] = o[t]
    return out


def run(volume, trace=False):
    """Returns (output, exec_time_ns_or_None)."""
    import concourse.bass_utils as bu
    from concourse.bass_utils import run_bass_kernel_spmd

    if trace:
        # avoid the S3 artifact upload in the axon trace path
        bu.upload_artifacts = lambda tmpdir: str(tmpdir)

    volume = np.asarray(volume, dtype=np.float32)
    M = build_M()
    in_maps = make_in_maps(volume, M)
    if "nc" not in _NC_CACHE:
        _NC_CACHE["nc"] = build_nc()
    nc = _NC_CACHE["nc"]
    res = run_bass_kernel_spmd(
        nc, in_maps, core_ids=list(range(NCORES)), trace=trace
    )
    out = gather_out(res.results)
    return out, getattr(res, "exec_time_ns", None)


def kernel(volume):
    out, _ = run(volume, trace=False)
    return out


# revision 32
# speedup vs baseline: 1.0382x; 1.0382x over previous
"""Trainium2 Bass kernel: cubic B-spline upsampling x2 of a (2,3,96,96,96) volume.

Math: the reference op (recursive IIR prefilter along each spatial axis, then
an 8-tap stride-2 transposed conv along each axis) is linear and separable.
The whole per-axis operator is a dense 192x96 matrix M (built exactly on the
host in float64).  out = M (x) M (x) M applied along z, y, x.

Device strategy (8 NeuronCores, SPMD, no collectives):
  24 tasks = 6 (b,c) volumes x 4 z'-slices of 48 rows; 3 tasks per core.
  Per task, three matmul stages in "data-stationary" form (stationary operand
  = data tile, moving operand = spline matrix), chosen so NO transposes are
  needed and the final stage lands with (z',y')-chunks on partitions and x'
  on the free axis == the DRAM output layout:
    A: per x  (96 mms): lhsT = vol[z, x, :]   (96z,128y-pad) rhs=MzT(96,48)  -> (y, z')
    B: per z' (48 mms): lhsT = L1[y, z', :]   (96y,128x-pad) rhs=MT (96,192) -> (x, y')
    C: per 128-chunk of (z',y') (72 mms): lhsT = L2[x, c] (96x,128) rhs=MT   -> (chunk, x')
  Compute in bf16 (PSUM accumulates fp32); volume is pre-cast/transposed on
  host; output written as fp32.
"""

import math
import os
import sys

import numpy as np

for _p in ("/opt/trn_rl_repo",):
    if _p not in sys.path and os.path.isdir(_p):
        sys.path.insert(0, _p)

import ml_dtypes  # noqa: E402

BF16 = ml_dtypes.bfloat16

POLE = math.sqrt(3.0) - 2.0
GAIN = (1.0 - POLE) * (1.0 - 1.0 / POLE)  # 6.0
N = 96
F = 2
NOUT = N * F  # 192
NTASK_PER_CORE = 3
NCORES = 8
ZSLICE = NOUT // 4  # 48


def _cubic(t):
    a = np.abs(t)
    out = (2.0 / 3.0 + (0.5 * a - 1.0) * a**2) * (a < 1)
    out = out + (-((a - 2.0) ** 3) / 6.0) * ((a >= 1) & (a < 2))
    return out


def _prefilter_mat(n):
    """96x96 matrix of the causal+anticausal cubic-spline prefilter (float64)."""
    p = POLE
    xm = np.eye(n, dtype=np.float64) * GAIN
    i = np.arange(n)
    pows = p**i + p ** (2 * n - 1 - i)
    c = np.zeros((n, n), dtype=np.float64)
    c[0] = (pows @ xm) * (p / (1.0 - p ** (2 * n))) + xm[0]
    for k in range(1, n):
        c[k] = xm[k] + p * c[k - 1]
    out = np.zeros((n, n), dtype=np.float64)
    out[n - 1] = c[n - 1] * (p / (p - 1.0))
    for k in range(n - 2, -1, -1):
        out[k] = p * (out[k + 1] - c[k])
    return out


def _upsample_mat(n, f=F):
    """2n x n matrix of the edge-padded stride-2 transposed conv (float64)."""
    k = 4 * f  # f even -> is_odd == 0
    start = 1.0 / (2 * f) - 2.0
    pts = np.arange(k, dtype=np.float64) * (1.0 / f) + start
    ker = _cubic(pts)
    npad = n + 4
    U = np.zeros((f * n, npad), dtype=np.float64)
    for o in range(f * n):
        for i in range(npad):
            s = o + (k - 1) - f * i
            if 0 <= s < k:
                U[o, i] += ker[s]
    Uc = np.zeros((f * n, n), dtype=np.float64)
    for i in range(npad):
        j = min(max(i - 2, 0), n - 1)
        Uc[:, j] += U[:, i]
    return Uc


def build_M():
    """Exact 192x96 per-axis operator (float64)."""
    return _upsample_mat(N) @ _prefilter_mat(N)


_NC_CACHE = {}


def _strip_redundant_self_waits(nc):
    """Drop sem waits that are trivially satisfied by same-engine program order.

    Tile's per-proc wait emission is not transitively minimal: a PE matmul can
    end up waiting on the PE's own semaphore (already guaranteed by in-order
    engine execution) in addition to a cross-engine wait, and the MM ISA
    struct only has one sync-wait slot (walrus: "Too many sync wait
    commands"). A wait on sem S is redundant for instruction I on engine E iff
    S is only ever updated by E and the cumulative updates to S from E before
    I already reach the wait value.
    """
    import concourse.mybir as mybir

    for fn in nc.m.functions:
        for blk in fn.blocks:
            updaters = {}  # sem id -> set of engines updating it (block-wide)
            for i in blk.instructions:
                si = i.sync_info
                if si is None:
                    continue
                for u in si.on_update or []:
                    updaters.setdefault(u.id, set()).add(i.engine)
            seen = {}  # (engine, sem id) -> cumulative update count so far
            for i in blk.instructions:
                si = i.sync_info
                if si is None:
                    continue
                if si.on_wait:
                    kept = []
                    for w in si.on_wait:
                        if (
                            w.sync_type == "semaphore"
                            and w.wait_mode == "sem-ge-imm"
                            and updaters.get(w.id) == {i.engine}
                            and seen.get((i.engine, w.id), 0) >= w.wait_value
                        ):
                            continue  # implied by program order
                        kept.append(w)
                    if len(kept) != len(si.on_wait):
                        si.on_wait[:] = kept
                for u in si.on_update or []:
                    key = (i.engine, u.id)
                    seen[key] = seen.get(key, 0) + u.update_value
            # each engine ISA struct has a single sync-wait slot: offload
            # extra waits onto same-engine nops inserted just before
            new_insts = []
            nop_n = 0
            for i in blk.instructions:
                si = i.sync_info
                if si is not None and si.on_wait and len(si.on_wait) > 1:
                    extra = list(si.on_wait[:-1])
                    si.on_wait[:] = [si.on_wait[-1]]
                    for w in extra:
                        nop = mybir.InstNoOp(
                            name=f"I-waitnop-{blk.name}-{nop_n}", ins=[], outs=[]
                        )
                        nop_n += 1
                        nop.engine = i.engine
                        nop.sync_info = mybir.SyncInfo(on_wait=[w], on_update=[])
                        new_insts.append(nop)
                new_insts.append(i)
            if nop_n:
                blk.instructions[:] = new_insts


def _hoist_input_dmas(nc, n_hoist=11):
    """Move the first input DMAs ahead of the sync engine's entry barrier.

    The Tile/BSP prologue (entry EVSEM barrier + TENSOR_LOAD) delays the
    first dma_start by ~7us. The leading input DMAs have no waits (inputs
    are resident at NEFF start, dst tiles untouched), so issuing them first
    starts the HBM reads during the prologue.
    """
    import concourse.mybir as mybir

    blocks = nc.m.functions[0].blocks
    body = blocks[1]
    dmas = []
    for i in body.instructions:
        if type(i).__name__ == "InstDMACopy" and i.engine == mybir.EngineType.SP:
            si = i.sync_info
            if si is not None and si.on_wait:
                break  # stop at the first gated DMA
            dmas.append(i)
            if len(dmas) >= n_hoist:
                break
    if not dmas:
        return
    dset = set(id(x) for x in dmas)
    body.instructions[:] = [i for i in body.instructions if id(i) not in dset]
    # insert into the prologue block after the leading InstCall, ahead of
    # the entry barrier: the sync engine starts immediately, so these DMAs
    # issue at t~0 while the other engines are still loading their code
    pro = blocks[0].instructions
    pos = 1 if pro and type(pro[0]).__name__ == "InstCall" else 0
    pro[:] = pro[:pos] + dmas + pro[pos:]


def build_nc():
    import concourse.bass as bass
    import concourse.mybir as mybir
    from concourse.tile import TileContext

    bf16 = mybir.dt.bfloat16
    f32 = mybir.dt.float32

    nc = bass.Bass(enable_partition_id=False)
    vol_ext = nc.declare_dram_parameter("vol", [3, 128, 96 * 128], bf16, isOutput=False)
    mzt_ext = nc.declare_dram_parameter("mzt", [3, 128, 48], bf16, isOutput=False)
    mt_ext = nc.declare_dram_parameter("mt", [128, 192], bf16, isOutput=False)
    out_ext = nc.declare_dram_parameter("out", [3 * 9216, 192], bf16, isOutput=True)


    with TileContext(nc) as tc:
        with (
            tc.tile_pool(name="consts", bufs=1) as consts,
            tc.tile_pool(name="vols", bufs=3) as vols_pool,
            tc.tile_pool(name="l1", bufs=2) as l1_pool,
            tc.tile_pool(name="l2", bufs=2) as l2_pool,
            tc.tile_pool(name="stage", bufs=5) as stage_pool,
            tc.tile_pool(name="pab", bufs=2, space="PSUM") as pab_pool,
            tc.tile_pool(name="pc", bufs=4, space="PSUM") as pc_pool,
        ):
            mt = consts.tile([128, 192], bf16)
            nc.sync.dma_start(out=mt[:], in_=mt_ext[:])
            mzt_all = consts.tile([128, 3, 48], bf16)
            nc.sync.dma_start(out=mzt_all[:], in_=mzt_ext[:].transpose([1, 0, 2]))

            # software-pipelined task loop: stage C of task t-1 finishes
            # interleaved with stage A of task t so out-DMAs keep flowing
            vols = []
            L1s = {}
            L2s = {}
            carry = None  # (emit_c_group fn, next group idx) of previous task

            def load_vol(t):
                vol = vols_pool.tile([128, 96 * 128], bf16, name="vol")
                # graduated chunks: small early ones so stage A never starves
                bounds = [0, 1024, 2048, 3072, 4608, 6656, 9472, 12288]
                for ch in range(7):
                    nc.sync.dma_start(
                        out=vol[:, bounds[ch] : bounds[ch + 1]],
                        in_=vol_ext[t, :, bounds[ch] : bounds[ch + 1]],
                    )
                return vol

            def make_emit_c(t, L2f):
                stage_tiles = {}

                # finer DMAs on the last quarter of the last task so the
                # final drain is short
                def _layout(g):
                    if t < 2 or g < 27:
                        return divmod(g, 9) + (9, 0)
                    if g < 33:
                        q2, gq = divmod(g - 27, 3)
                        return 3 + q2, gq, 3, 6912
                    return 5 + (g - 33), 0, 1, 8448

                def emit_c_group(g):
                    q, gq, gper, base = _layout(g)
                    if gq == 0:
                        stage_tiles[q] = stage_pool.tile(
                            [128, 2 * gper, 192], bf16, name="stage"
                        )
                    stage = stage_tiles[q]
                    pc = pc_pool.tile([128, 2, 192], f32, name="pc")
                    for j in range(2):
                        ch = g * 2 + j
                        nc.tensor.matmul(
                            pc[:, j, :],
                            lhsT=L2f[:, ch * 128 : (ch + 1) * 128],
                            rhs=mt[:],
                            start=True,
                            stop=True,
                        )
                    if g in (17, 35):
                        nc.vector.tensor_copy(
                            stage[:, gq * 2 : gq * 2 + 2, :], pc[:, :, :]
                        )
                    else:
                        nc.scalar.copy(
                            stage[:, gq * 2 : gq * 2 + 2, :], pc[:, :, :]
                        )
                    if gq == gper - 1:
                        rows = 256 * gper
                        qoff = 0 if not base else (3 if gper == 3 else 5)
                        r0 = t * 9216 + base + (q - qoff) * rows
                        nc.sync.dma_start(
                            out=out_ext[r0 : r0 + rows, :].rearrange(
                                "(cl p) x -> p cl x", p=128
                            ),
                            in_=stage[:],
                        )

                return emit_c_group

            vols = [load_vol(0), load_vol(1), load_vol(2)]

            for t in range(3):
                vol = vols[t]

                # ---- stage A: contract z -> (y, z') per x ----
                # L1 layout (y, x, z'): stage-A copies are fully contiguous;
                # stage B's stationary reads x strided (LDW-side cost only)
                # L1 layout (y, z', x-pad128): B's stationary reads 128
                # contiguous cols; stage-A matmuls write PSUM strided
                # (x-minor) to match. y-rows 96:128 arrive as zeros via the
                # copies (zero-padded stationaries), only x-cols need memset.
                L1 = l1_pool.tile([128, 48, 128], bf16)
                nc.gpsimd.memset(L1[:, :, 96:128], 0.0)
                for g in range(12):  # groups of 8 x-slices -> one PSUM bank
                    pa = pab_pool.tile([128, 48, 8], f32, name="pa", tag="pab")
                    for j in range(8):
                        x = g * 8 + j
                        nc.tensor.matmul(
                            pa[:, :, j],
                            lhsT=vol[:, x * 128 : (x + 1) * 128],
                            rhs=mzt_all[:, t, :],
                            start=True,
                            stop=True,
                        )
                    # psum (96p, z':48, x:8) -> L1 (96p, z':48, x:8 at offset)
                    nc.vector.tensor_copy(
                        L1[:, :, g * 8 : (g + 1) * 8],
                        pa[:, :, :],
                    )
                    # previous task's held-back C groups ride along with A
                    if carry is not None:
                        c_emit, c_next = carry
                        c_emit(c_next)
                        carry = (c_emit, c_next + 1) if c_next + 1 < 36 else None
                while carry is not None:
                    c_emit, c_next = carry
                    c_emit(c_next)
                    carry = (c_emit, c_next + 1) if c_next + 1 < 36 else None

                # ---- stages B and C, interleaved ----
                # B (contract y -> (x, y') per z') fills L2 z'-rows in order;
                # C chunk c only needs z' <= ((c+1)*128-1)//192, so C groups
                # are emitted as soon as their rows exist; the last 4 groups
                # are held back to overlap the next task's stage A.
                L2 = l2_pool.tile([128, 48, 192], bf16)
                L2f = L2[:].rearrange("p a b -> p (a b)")  # (128, 9216)
                emit_c = make_emit_c(t, L2f)
                hold = 9 if t < 2 else 0
                g_next = 0
                for zz in range(12):  # B groups of 4 z' (2 PSUM banks)
                    pb = pab_pool.tile(
                        [128, 2, 2, 192], f32, name="pb", tag="pab",
                        padded_shape=[128, 2, 2, 256],
                    )
                    for b in range(2):
                        for jj in range(2):
                            zp = zz * 4 + b * 2 + jj
                            nc.tensor.matmul(
                                pb[:, b, jj, :],
                                lhsT=L1[:, zp, :],
                                rhs=mt[:],
                                start=True,
                                stop=True,
                            )
                    nc.vector.tensor_copy(
                        L2[:, zz * 4 : zz * 4 + 4, :].rearrange(
                            "p (b j) y -> p b j y", b=2
                        ),
                        pb[:, :, :, :],
                    )
                    rows_done = (zz + 1) * 4 * 192
                    while (
                        g_next < 36 - hold and (g_next * 2 + 2) * 128 <= rows_done
                    ):
                        emit_c(g_next)
                        g_next += 1
                while g_next < 36 - hold:
                    emit_c(g_next)
                    g_next += 1
                carry = (emit_c, g_next) if hold else None
    _strip_redundant_self_waits(nc)
    _hoist_input_dmas(nc)
    return nc


def _task_map(core, t):
    gt = NTASK_PER_CORE * core + t
    bc, s = divmod(gt, 4)
    b, c = divmod(bc, 3)
    return b, c, s


def make_in_maps(volume, M):
    mt_b = np.zeros((128, 192), dtype=BF16)  # K-padded to 128 rows
    mt_b[:96] = np.ascontiguousarray(M.T).astype(BF16)
    in_maps = []
    for core in range(NCORES):
        vols = np.zeros((3, 128, 96, 128), dtype=BF16)
        mzts = np.zeros((3, 128, 48), dtype=BF16)
        for t in range(NTASK_PER_CORE):
            b, c, s = _task_map(core, t)
            vt = np.transpose(volume[b, c], (0, 2, 1))  # (z, x, y)
            vols[t, :96, :, :96] = vt.astype(BF16)
            mzts[t, :96] = np.ascontiguousarray(
                M[s * ZSLICE : (s + 1) * ZSLICE, :].T
            ).astype(BF16)
        in_maps.append(
            {"vol": vols.reshape(3, 128, 96 * 128), "mzt": mzts, "mt": mt_b}
        )
    return in_maps


def gather_out(results):
    out = np.zeros((2, 3, 192, 192, 192), dtype=np.float32)
    for core in range(NCORES):
        o = np.asarray(results[core]["out"], dtype=np.float32).reshape(3, 48, 192, 192)
        for t in range(NTASK_PER_CORE):
            b, c, s = _task_map(core, t)
            out[b, c, s * ZSLICE : (s + 1) * ZSLICE] = o[t]
    return out


def run(volume, trace=False):
    """Returns (output, exec_time_ns_or_None)."""
    import concourse.bass_utils as bu
    from concourse.bass_utils import run_bass_kernel_spmd

    if trace:
        # avoid the S3 artifact upload in the axon trace path
        bu.upload_artifacts = lambda tmpdir: str(tmpdir)

    volume = np.asarray(volume, dtype=np.float32)
    M = build_M()
    in_maps = make_in_maps(volume, M)
    if "nc" not in _NC_CACHE:
        _NC_CACHE["nc"] = build_nc()
    nc = _NC_CACHE["nc"]
    res = run_bass_kernel_spmd(
        nc, in_maps, core_ids=list(range(NCORES)), trace=trace
    )
    out = gather_out(res.results)
    return out, getattr(res, "exec_time_ns", None)


def kernel(volume):
    out, _ = run(volume, trace=False)
    return out
